# revision 4
# baseline (speedup 1.0000x reference)
"""Trainium2 Bass kernel for CoordsSelect (batched voxel-feature gather), v3.

reference semantics:
  volume: [B=4, F=16, D=120, D, D] f32, coords: [B, 3*A=6144] f32,
  num_atoms: [B] int32
  vox = floor(coords_xyz) (clipped to [0,119]); flat = ix*D*D + iy*D + iz
  out[b, f, a] = volume[b, f].flat[flat[b, a]] * (a < num_atoms[b])

Sharding: 8 cores = 4 batches x 2 feature-halves. Core c handles
batch c//2, features 8*(c%2) .. 8*(c%2)+8, all 2048 atoms.

Volume is relaid out HOST-side to window-major bf16
  vol_wm[w, f, v] = volume[b, fh*8+f, 64*w + v],  w in [0, 27000)
so ONE gather descriptor (elem 8*64 bf16 = 1KB) fetches ALL 8 features'
64-voxel windows for an atom: 2048 descriptors total instead of 8*2048,
and half the HBM bytes. bf16 rounding (~2^-9 rel) is far inside the 2e-2
tolerance and masked atoms stay exactly 0.

Per-core algorithm (all on device):
  1. flat voxel ids: fused floor chain on the [128, 384] chunk-layout
     coords (robust to cast rounding mode), then
     flat = reduce_add(floor(c) * [14400, 120, 1]). A separate small
     [128, 48] chain (gather-output layout) feeds the within-window
     selector so the idx path only waits on the first coords DMA.
  2. idxs = (flat >> 6) permuted to the dma_gather wrap order, int16.
  3. 4 chunked dma_gathers (512 idxs each) in prepare_only mode +
     trigger_dma on round-robin SWDGE queues: the Pool engine only pays
     descriptor GENERATION (~0.3ns/desc); transfers fly async and
     pipeline against the DVE select. An explicit wait_ge(dma_sem, 16)
     on the Vector engine gates each select on DMA completion (Tile's
     auto-wait under-synchronizes for prepare_only SBUF readers).
  4. per chunk: sel = g * onehot(within) (TT mult, 4x mode bf16), then a
     TT-add halving prepass (64->32) and a reduce over 32 -> res f32.
     Invalid atoms (a >= num_atoms) get their selector pushed out of
     [0,64) so they produce exact 0.
  5. per-chunk [128, 4, 8] f32 DMA writes; the host unscrambles the
     (p, j) -> atom order afterwards.

dma_gather index wrap (per HW/ucode semantics): index position i lives at
idxs[i % 16, i // 16] (replicated across the 8 16-partition groups), and
gather output row i lands at out[i % 128, i // 128, :]. We assign position
i the atom a(i) = (i%16)*128 + ((i%128)//16)*16 + (i//128), which makes:
  - idxs[p, c] = w_tile[p, (c%8)*16 + c//8]   (pure free-dim permutation of
    the natural chunk-per-partition tile w_tile[p, m] = w(atom (p%16)*128+m))
  - gather out[p, j] = atom base(p) + j with base(p) = (p%16)*128+(p//16)*16
    i.e. 16 consecutive atoms per partition -> the within-window selector
    comes from one contiguous coords re-load.
"""

import os

import numpy as np

import concourse.bass as bass
import concourse.mybir as mybir
import concourse.tile as tile
from concourse import bacc, library_config
from concourse.bass_utils import run_bass_kernel_spmd

B, F, D = 4, 16, 120
A = 2048
D3 = D * D * D          # 1_728_000
FC = F // 2             # 8 features per core
NROWS = D3 // 64        # 27_000 window rows per (batch, feature-half)
ELEM = FC * 64          # 512 bf16 = 1KB per gather descriptor
N_CORES = 8

# gather pipelining config (env-tweakable for experiments)
PREPARE = os.environ.get("CS_PREPARE", "1") == "1"
NCHUNKS = int(os.environ.get("CS_NCHUNKS", "4"))
NQUEUES = int(os.environ.get("CS_NQUEUES", "4"))
CHUNK = A // NCHUNKS            # idxs per gather call
JC = CHUNK // 128               # j-slots per chunk

f32 = mybir.dt.float32
bf16 = mybir.dt.bfloat16
i32 = mybir.dt.int32
i16 = mybir.dt.int16
Alu = mybir.AluOpType
AxisX = mybir.AxisListType.X


def _floor(nc, pool, src, n, tag):
    """fx = floor(src) for src >= 0, robust to the f32->i32 cast rounding
    mode: i = int(x); f = float(i); fx = f - (f > x)."""
    ti = pool.tile([128, n], i32, name=f"ti{tag}")
    tf = pool.tile([128, n], f32, name=f"tf{tag}")
    gt = pool.tile([128, n], f32, name=f"gt{tag}")
    fx = pool.tile([128, n], f32, name=f"fx{tag}")
    nc.vector.tensor_copy(out=ti[:], in_=src)
    nc.vector.tensor_copy(out=tf[:], in_=ti[:])
    nc.vector.tensor_tensor(out=gt[:], in0=tf[:], in1=src, op=Alu.is_gt)
    nc.vector.tensor_tensor(out=fx[:], in0=tf[:], in1=gt[:], op=Alu.subtract)
    return fx


def _flat(nc, pool, fx, w3_t, n, tag):
    """flat_i[p, a] = int(fx[p, 3a]*D*D + fx[p, 3a+1]*D + fx[p, 3a+2])."""
    wprod = pool.tile([128, n, 3], f32, name=f"wp{tag}")
    nc.vector.tensor_tensor(
        out=wprod[:],
        in0=fx[:].rearrange("p (a d) -> p a d", d=3),
        in1=w3_t[:].rearrange("p (x d) -> p x d", x=1).to_broadcast([128, n, 3]),
        op=Alu.mult,
    )
    flat_f = pool.tile([128, n], f32, name=f"ff{tag}")
    nc.vector.tensor_reduce(out=flat_f[:], in_=wprod[:], axis=AxisX, op=Alu.add)
    flat_i = pool.tile([128, n], i32, name=f"fi{tag}")
    nc.vector.tensor_copy(out=flat_i[:], in_=flat_f[:])
    return flat_i


def build_bass(debug_dumps=False):
    """Build + compile the per-core Bass program (identical on all cores)."""
    nc = bacc.Bacc(
        "TRN2",
        target_bir_lowering=False,
        debug=False,
        num_devices=N_CORES,
        num_swdge_queues=NQUEUES,
    )

    vol = nc.dram_tensor("vol", [NROWS * ELEM], bf16, kind="ExternalInput")
    crd = nc.dram_tensor("crd", [3 * A], f32, kind="ExternalInput")
    nat = nc.dram_tensor("nat", [128], i32, kind="ExternalInput")
    # host-provided constants: atom ids in gather-output layout, the xyz
    # combine weights, and the 0..63 window ramp
    am0 = nc.dram_tensor("am0", [128, 16], i32, kind="ExternalInput")
    w3c = nc.dram_tensor("w3c", [128, 3], f32, kind="ExternalInput")
    cec = nc.dram_tensor("cec", [128, 64], i32, kind="ExternalInput")
    out = nc.dram_tensor("out", [128, 16, FC], f32, kind="ExternalOutput")

    with tile.TileContext(nc) as tc:
        with (
            tc.tile_pool(name="p", bufs=1) as pool,
            tc.tile_pool(name="gp", bufs=3) as gpool,
            tc.tile_pool(name="sp", bufs=2) as spool,
        ):
            # dma_gather lives in the 'mlp' Q7 ucode library
            nc.gpsimd.load_library(library_config.mlp)

            # chunk-layout coords (idx path): partition p holds the 128
            # atoms of chunk p%16 (replicated across the 8 groups)
            crd_t = pool.tile([128, 384], f32)
            nc.sync.dma_start(
                crd_t[:], bass.AP(crd, 0, [[0, 8], [384, 16], [1, 384]])
            )
            # gather-output-layout coords (within path): partition p holds
            # the 16 consecutive atoms starting at base(p)
            crd2_t = pool.tile([128, 48], f32)
            nc.scalar.dma_start(
                crd2_t[:], bass.AP(crd, 0, [[48, 8], [384, 16], [1, 48]])
            )

            w3_t = pool.tile([128, 3], f32)
            nc.sync.dma_start(w3_t[:], w3c.ap())
            am0_t = pool.tile([128, 16], i32)
            nc.scalar.dma_start(am0_t[:], am0.ap())
            nat_t = pool.tile([128, 1], i32)
            nc.scalar.dma_start(nat_t[:], nat.ap()[:, None])
            ce_t = pool.tile([128, 64], i32)
            nc.scalar.dma_start(ce_t[:], cec.ap())

            # ---- idx path: only gated by the crd_t DMA ----
            fx1 = _floor(nc, pool, crd_t[:], 384, "a")
            fl1 = _flat(nc, pool, fx1, w3_t, 128, "a")
            w_i = pool.tile([128, 128], i32)
            nc.vector.tensor_scalar(
                w_i[:], fl1[:], 6, None, op0=Alu.arith_shift_right
            )
            idxs = pool.tile([128, 128], i16)
            nc.vector.tensor_copy(
                out=idxs[:].rearrange("p (ch c8) -> p ch c8", c8=8),
                in_=w_i[:].rearrange("p (c8 ch) -> p ch c8", c8=8),
            )

            # ---- within path (overlaps the first gathers) ----
            fx2 = _floor(nc, pool, crd2_t[:], 48, "b")
            fl2 = _flat(nc, pool, fx2, w3_t, 16, "b")
            win_i = pool.tile([128, 16], i32)
            nc.vector.tensor_scalar(
                win_i[:], fl2[:], 63, None, op0=Alu.bitwise_and
            )
            pen = pool.tile([128, 16], i32)
            nc.vector.tensor_tensor(
                out=pen[:],
                in0=am0_t[:],
                in1=nat_t[:].to_broadcast([128, 16]),
                op=Alu.is_ge,
            )
            win2 = pool.tile([128, 16], i32)
            nc.vector.scalar_tensor_tensor(
                out=win2[:],
                in0=pen[:],
                scalar=65,
                in1=win_i[:],
                op0=Alu.mult,
                op1=Alu.add,
            )
            # one-hot selector oh[p, j, v] = (v == win2[p, j]), bf16
            oh = pool.tile([128, 16, 64], bf16)
            nc.vector.tensor_tensor(
                out=oh[:],
                in0=ce_t[:]
                .rearrange("p (x v) -> p x v", x=1)
                .to_broadcast([128, 16, 64]),
                in1=win2[:]
                .rearrange("p (j x) -> p j x", x=1)
                .to_broadcast([128, 16, 64]),
                op=Alu.is_equal,
            )

            # ---- chunked gather + select ----
            in_ap = bass.AP(vol, 0, [[ELEM, NROWS], [1, ELEM]])
            nc16 = CHUNK // 16
            for ci in range(NCHUNKS):
                q = ci % NQUEUES
                g_out = gpool.tile([128, JC, ELEM], bf16, name="g_out")
                if PREPARE:
                    dma_sem = nc.alloc_semaphore(f"gsem{ci}")
                    nc.gpsimd.dma_gather(
                        out_ap=g_out[:],
                        in_ap=in_ap,
                        idxs_ap=idxs[:, ci * nc16 : (ci + 1) * nc16],
                        num_idxs=CHUNK,
                        num_idxs_reg=CHUNK,
                        elem_size=ELEM,
                        prepare_only=True,
                        sem=dma_sem,
                        single_packet=False,
                        queue_num=q,
                    )
                    nc.gpsimd.trigger_dma(count=None, queue_num=q)
                    # Tile's auto-wait for prepare_only SBUF readers is
                    # threshold-0; gate the select explicitly on the DMA sem.
                    nc.vector.wait_ge(dma_sem, 16)
                else:
                    nc.gpsimd.dma_gather(
                        out_ap=g_out[:],
                        in_ap=in_ap,
                        idxs_ap=idxs[:, ci * nc16 : (ci + 1) * nc16],
                        num_idxs=CHUNK,
                        num_idxs_reg=CHUNK,
                        elem_size=ELEM,
                        single_packet=False,
                        queue_num=q,
                    )
                sel = spool.tile([128, JC, FC, 64], bf16, name="sel")
                nc.vector.tensor_tensor(
                    out=sel[:],
                    in0=g_out[:].rearrange("p j (f v) -> p j f v", v=64),
                    in1=oh[:, ci * JC : (ci + 1) * JC, :]
                    .rearrange("p j (x v) -> p j x v", x=1)
                    .to_broadcast([128, JC, FC, 64]),
                    op=Alu.mult,
                )
                # halving prepass keeps the expensive pass in TT 4x mode;
                # the final reduce (2x ceiling with f32 out) sees half the work
                sh = spool.tile([128, JC, FC, 32], bf16, name="sh")
                nc.vector.tensor_tensor(
                    out=sh[:],
                    in0=sel[:, :, :, 0:32],
                    in1=sel[:, :, :, 32:64],
                    op=Alu.add,
                )
                res = spool.tile([128, JC, FC], f32, name="res")
                nc.vector.tensor_reduce(
                    out=res[:], in_=sh[:], axis=AxisX, op=Alu.add
                )
                eng = nc.sync if ci % 2 == 0 else nc.scalar
                eng.dma_start(
                    bass.AP(out, ci * JC * FC, [[16 * FC, 128], [1, JC * FC]]),
                    res[:],
                )

            if debug_dumps:
                d_idxs = nc.dram_tensor(
                    "d_idxs", [128, 128], i16, kind="ExternalOutput"
                )
                nc.sync.dma_start(d_idxs.ap(), idxs[:])
                d_win2 = nc.dram_tensor(
                    "d_win2", [128, 16], i32, kind="ExternalOutput"
                )
                nc.sync.dma_start(d_win2.ap(), win2[:])

    nc.compile()
    return nc


_NC_CACHE = None


def _get_nc():
    global _NC_CACHE
    if _NC_CACHE is None:
        _NC_CACHE = build_bass()
    return _NC_CACHE


def _base_p():
    p = np.arange(128)
    return (p % 16) * 128 + (p // 16) * 16


def _consts():
    base = _base_p()
    am0 = (base[:, None] + np.arange(16)[None, :]).astype(np.int32)
    w3 = np.tile(
        np.array([D * D, D, 1], dtype=np.float32)[None, :], (128, 1)
    )
    ce = np.tile(np.arange(64, dtype=np.int32)[None, :], (128, 1))
    return am0, w3, ce


# atom id for result slot (p, j): ATOM_ORDER[p*16 + j] = base(p) + j
ATOM_ORDER = (_base_p()[:, None] + np.arange(16)[None, :]).reshape(-1)


def unscramble(res):
    """res: [128, 16, FC] device result -> [FC, A] in atom order."""
    oc = np.empty((FC, A), dtype=np.float32)
    oc[:, ATOM_ORDER] = np.asarray(res).transpose(2, 0, 1).reshape(FC, A)
    return oc


def make_in_maps(volume, coords, num_atoms):
    import ml_dtypes

    am0, w3, ce = _consts()
    in_maps = []
    for c in range(N_CORES):
        b, fh = c // 2, c % 2
        v = volume[b, fh * FC : (fh + 1) * FC].reshape(FC, NROWS, 64)
        v = v.transpose(1, 0, 2).astype(ml_dtypes.bfloat16)  # [w, f, v]
        in_maps.append(
            {
                "vol": np.ascontiguousarray(v).reshape(-1),
                "crd": np.ascontiguousarray(coords[b]),
                "nat": np.full((128,), num_atoms[b], dtype=np.int32),
                "am0": am0,
                "w3c": w3,
                "cec": ce,
            }
        )
    return in_maps


def kernel(volume, coords, num_atoms):
    volume = np.asarray(volume, dtype=np.float32)
    coords = np.asarray(coords, dtype=np.float32)
    num_atoms = np.asarray(num_atoms, dtype=np.int32)

    nc = _get_nc()
    in_maps = make_in_maps(volume, coords, num_atoms)
    r = run_bass_kernel_spmd(nc, in_maps, core_ids=list(range(N_CORES)))

    out = np.empty((B, F, A), dtype=np.float32)
    for c, res in enumerate(r.results):
        b, fh = c // 2, c % 2
        out[b, fh * FC : (fh + 1) * FC] = unscramble(res["out"])
    return out


# revision 7
# speedup vs baseline: 1.2765x; 1.2765x over previous
"""Trainium2 Bass kernel for CoordsSelect (batched voxel-feature gather), v4.

reference semantics:
  volume: [B=4, F=16, D=120, D, D] f32, coords: [B, 3*A=6144] f32,
  num_atoms: [B] int32
  vox = floor(coords_xyz) (clipped to [0,119]); flat = ix*D*D + iy*D + iz
  out[b, f, a] = volume[b, f].flat[flat[b, a]] * (a < num_atoms[b])

Sharding (v4): 8 cores = 4 batches x 2 ATOM-halves. Core c handles
batch c//2, atoms 1024*(c%2) .. 1024*(c%2)+1024, ALL 16 features.

Volume is relaid out HOST-side to window-major bf16
  vol_wm[w, f, v] = volume[b, f, 64*w + v],  w in [0, 27000)
so ONE gather descriptor (elem 16*64 bf16 = 2KB) fetches ALL 16 features'
64-voxel windows for an atom. The Q7 gather ucode costs ~8.3ns per
descriptor regardless of element size (measured; descriptor processing,
not bandwidth, is the bottleneck), so descriptors/core is the metric:
v1 did 8*2048 = 16384, v4 does 1024. bf16 rounding (~2^-9 rel) is far
inside the 2e-2 tolerance and masked atoms stay exactly 0.

Per-core algorithm (all on device):
  1. flat voxel ids: fused floor chain on the [128, 192] chunk-layout
     coords (robust to cast rounding mode), then
     flat = reduce_add(floor(c) * [14400, 120, 1]). The coords DMA is
     issued from the Vector queue so the chain isn't gated on
     cross-engine DMA semaphores. A separate [128, 24] chain
     (gather-output layout) feeds the within-window selector.
  2. idxs = (flat >> 6) permuted to the dma_gather wrap order, int16.
  3. 2 chunked blocking dma_gathers (512 idxs each) on the Pool engine;
     chunk 1 gathers while the DVE selects chunk 0.
  4. per chunk: sel = g * onehot(within) (TT mult, 4x mode bf16), then a
     TT-add halving prepass (64->32, still 4x) and a reduce over 32
     (2x ceiling with f32 out) -> res f32. Invalid atoms (a >=
     num_atoms) get their selector pushed out of [0,64) -> exact 0.
  5. per-chunk [128, 4, 16] f32 DMA writes; the host unscrambles the
     (p, j) -> atom order afterwards.

dma_gather index wrap (per HW/ucode semantics): index position i lives at
idxs[i % 16, i // 16] (replicated across the 8 16-partition groups), and
gather output row i lands at out[i % 128, i // 128, :]. With MC = 64
atoms per partition-chunk and JT = 8 j-slots, we assign position i the
atom a(i) = (i%16)*MC + JT*((i//16)%8) + (i//128), which makes:
  - idxs[p, cc] = w_tile[p, JT*(cc%8) + cc//8]  (pure free-dim
    permutation of the natural chunk-per-partition tile
    w_tile[p, m] = w(atom (p%16)*MC + m))
  - gather out[p, j] = atom base(p) + j, base(p) = (p%16)*MC + (p//16)*JT
    i.e. 8 consecutive atoms per partition -> the within-window selector
    comes from one contiguous coords re-load.
"""

import os

import numpy as np

import concourse.bass as bass
import concourse.mybir as mybir
import concourse.tile as tile
from concourse import bacc, library_config
from concourse.bass_utils import run_bass_kernel_spmd

B, F, D = 4, 16, 120
A = 2048
D3 = D * D * D          # 1_728_000
AC = A // 2             # 1024 atoms per core
MC = AC // 16           # 64 atoms per partition-chunk
JT = AC // 128          # 8 j-slots
NROWS = D3 // 64        # 27_000 window rows per batch
ELEM = F * 64           # 1024 bf16 = 2KB per gather descriptor
N_CORES = 8

PREPARE = os.environ.get("CS_PREPARE", "0") == "1"
NCHUNKS = int(os.environ.get("CS_NCHUNKS", "2"))
NQUEUES = int(os.environ.get("CS_NQUEUES", "1"))
CHUNK = AC // NCHUNKS           # idxs per gather call
JC = CHUNK // 128               # j-slots per chunk

f32 = mybir.dt.float32
bf16 = mybir.dt.bfloat16
i32 = mybir.dt.int32
i16 = mybir.dt.int16
Alu = mybir.AluOpType
AxisX = mybir.AxisListType.X


def _floor(nc, pool, src, n, tag):
    """fx = floor(src) for src >= 0, robust to the f32->i32 cast rounding
    mode: i = int(x); f = float(i); fx = f - (f > x)."""
    ti = pool.tile([128, n], i32, name=f"ti{tag}")
    tf = pool.tile([128, n], f32, name=f"tf{tag}")
    gt = pool.tile([128, n], f32, name=f"gt{tag}")
    fx = pool.tile([128, n], f32, name=f"fx{tag}")
    nc.vector.tensor_copy(out=ti[:], in_=src)
    nc.vector.tensor_copy(out=tf[:], in_=ti[:])
    nc.vector.tensor_tensor(out=gt[:], in0=tf[:], in1=src, op=Alu.is_gt)
    nc.vector.tensor_tensor(out=fx[:], in0=tf[:], in1=gt[:], op=Alu.subtract)
    return fx


def _flat(nc, pool, fx, w3_t, n, tag):
    """flat_i[p, a] = int(fx[p, 3a]*D*D + fx[p, 3a+1]*D + fx[p, 3a+2])."""
    wprod = pool.tile([128, n, 3], f32, name=f"wp{tag}")
    nc.vector.tensor_tensor(
        out=wprod[:],
        in0=fx[:].rearrange("p (a d) -> p a d", d=3),
        in1=w3_t[:].rearrange("p (x d) -> p x d", x=1).to_broadcast([128, n, 3]),
        op=Alu.mult,
    )
    flat_f = pool.tile([128, n], f32, name=f"ff{tag}")
    nc.vector.tensor_reduce(out=flat_f[:], in_=wprod[:], axis=AxisX, op=Alu.add)
    flat_i = pool.tile([128, n], i32, name=f"fi{tag}")
    nc.vector.tensor_copy(out=flat_i[:], in_=flat_f[:])
    return flat_i


def build_bass(debug_dumps=False):
    """Build + compile the per-core Bass program (identical on all cores)."""
    nc = bacc.Bacc(
        "TRN2",
        target_bir_lowering=False,
        debug=False,
        num_devices=N_CORES,
        num_swdge_queues=NQUEUES,
    )

    vol = nc.dram_tensor("vol", [NROWS * ELEM], bf16, kind="ExternalInput")
    crd = nc.dram_tensor("crd", [3 * AC], f32, kind="ExternalInput")
    nat = nc.dram_tensor("nat", [128], i32, kind="ExternalInput")
    am0 = nc.dram_tensor("am0", [128, JT], i32, kind="ExternalInput")
    w3c = nc.dram_tensor("w3c", [128, 3], f32, kind="ExternalInput")
    cec = nc.dram_tensor("cec", [128, 64], i32, kind="ExternalInput")
    out = nc.dram_tensor("out", [128, JT, F], f32, kind="ExternalOutput")

    with tile.TileContext(nc) as tc:
        with (
            tc.tile_pool(name="p", bufs=1) as pool,
            tc.tile_pool(name="gp", bufs=2) as gpool,
            tc.tile_pool(name="sp", bufs=2) as spool,
        ):
            # dma_gather lives in the 'mlp' Q7 ucode library
            nc.gpsimd.load_library(library_config.mlp)

            # chunk-layout coords (idx path): partition p holds the MC=64
            # atoms of chunk p%16 (replicated across the 8 groups). Issued
            # from the Vector queue (floor chain is the critical path).
            crd_t = pool.tile([128, 3 * MC], f32)
            nc.sync.dma_start(
                crd_t[:], bass.AP(crd, 0, [[0, 8], [3 * MC, 16], [1, 3 * MC]])
            )
            # gather-output-layout coords (within path): partition p holds
            # the JT=8 consecutive atoms starting at base(p)
            crd2_t = pool.tile([128, 3 * JT], f32)
            nc.scalar.dma_start(
                crd2_t[:],
                bass.AP(crd, 0, [[3 * JT, 8], [3 * MC, 16], [1, 3 * JT]]),
            )

            w3_t = pool.tile([128, 3], f32)
            nc.sync.dma_start(w3_t[:], w3c.ap())
            am0_t = pool.tile([128, JT], i32)
            nc.scalar.dma_start(am0_t[:], am0.ap())
            nat_t = pool.tile([128, 1], i32)
            nc.scalar.dma_start(nat_t[:], nat.ap()[:, None])
            ce_t = pool.tile([128, 64], i32)
            nc.scalar.dma_start(ce_t[:], cec.ap())

            # ---- idx path ----
            fx1 = _floor(nc, pool, crd_t[:], 3 * MC, "a")
            fl1 = _flat(nc, pool, fx1, w3_t, MC, "a")
            w_i = pool.tile([128, MC], i32)
            nc.vector.tensor_scalar(
                w_i[:], fl1[:], 6, None, op0=Alu.arith_shift_right
            )
            idxs = pool.tile([128, MC], i16)
            nc.vector.tensor_copy(
                out=idxs[:].rearrange("p (ch c8) -> p ch c8", c8=8),
                in_=w_i[:].rearrange("p (c8 ch) -> p ch c8", c8=8),
            )

            # ---- within path (overlaps the first gather) ----
            fx2 = _floor(nc, pool, crd2_t[:], 3 * JT, "b")
            fl2 = _flat(nc, pool, fx2, w3_t, JT, "b")
            win_i = pool.tile([128, JT], i32)
            nc.vector.tensor_scalar(
                win_i[:], fl2[:], 63, None, op0=Alu.bitwise_and
            )
            pen = pool.tile([128, JT], i32)
            nc.vector.tensor_tensor(
                out=pen[:],
                in0=am0_t[:],
                in1=nat_t[:].to_broadcast([128, JT]),
                op=Alu.is_ge,
            )
            win2 = pool.tile([128, JT], i32)
            nc.vector.scalar_tensor_tensor(
                out=win2[:],
                in0=pen[:],
                scalar=65,
                in1=win_i[:],
                op0=Alu.mult,
                op1=Alu.add,
            )
            # one-hot selector oh[p, j, v] = (v == win2[p, j]), bf16
            oh = pool.tile([128, JT, 64], bf16)
            nc.vector.tensor_tensor(
                out=oh[:],
                in0=ce_t[:]
                .rearrange("p (x v) -> p x v", x=1)
                .to_broadcast([128, JT, 64]),
                in1=win2[:]
                .rearrange("p (j x) -> p j x", x=1)
                .to_broadcast([128, JT, 64]),
                op=Alu.is_equal,
            )

            # ---- chunked gather + select ----
            in_ap = bass.AP(vol, 0, [[ELEM, NROWS], [1, ELEM]])
            nc16 = CHUNK // 16
            for ci in range(NCHUNKS):
                q = ci % NQUEUES
                g_out = gpool.tile([128, JC, ELEM], bf16, name="g_out")
                if PREPARE:
                    dma_sem = nc.alloc_semaphore(f"gsem{ci}")
                    nc.gpsimd.dma_gather(
                        out_ap=g_out[:],
                        in_ap=in_ap,
                        idxs_ap=idxs[:, ci * nc16 : (ci + 1) * nc16],
                        num_idxs=CHUNK,
                        num_idxs_reg=CHUNK,
                        elem_size=ELEM,
                        prepare_only=True,
                        sem=dma_sem,
                        single_packet=False,
                        queue_num=q,
                    )
                    nc.gpsimd.trigger_dma(count=None, queue_num=q)
                    nc.vector.wait_ge(dma_sem, 16)
                else:
                    nc.gpsimd.dma_gather(
                        out_ap=g_out[:],
                        in_ap=in_ap,
                        idxs_ap=idxs[:, ci * nc16 : (ci + 1) * nc16],
                        num_idxs=CHUNK,
                        num_idxs_reg=CHUNK,
                        elem_size=ELEM,
                        single_packet=False,
                        queue_num=q,
                    )
                sel = spool.tile([128, JC, F, 64], bf16, name="sel")
                nc.vector.tensor_tensor(
                    out=sel[:],
                    in0=g_out[:].rearrange("p j (f v) -> p j f v", v=64),
                    in1=oh[:, ci * JC : (ci + 1) * JC, :]
                    .rearrange("p j (x v) -> p j x v", x=1)
                    .to_broadcast([128, JC, F, 64]),
                    op=Alu.mult,
                )
                # halving prepass keeps the expensive pass in TT 4x mode;
                # the final reduce (2x ceiling, f32 out) sees half the work
                sh = spool.tile([128, JC, F, 32], bf16, name="sh")
                nc.vector.tensor_tensor(
                    out=sh[:],
                    in0=sel[:, :, :, 0:32],
                    in1=sel[:, :, :, 32:64],
                    op=Alu.add,
                )
                res = spool.tile([128, JC, F], f32, name="res")
                nc.vector.tensor_reduce(
                    out=res[:], in_=sh[:], axis=AxisX, op=Alu.add
                )
                eng = nc.sync if ci % 2 == 0 else nc.scalar
                eng.dma_start(
                    bass.AP(out, ci * JC * F, [[JT * F, 128], [1, JC * F]]),
                    res[:],
                )

            if debug_dumps:
                d_idxs = nc.dram_tensor(
                    "d_idxs", [128, MC], i16, kind="ExternalOutput"
                )
                nc.sync.dma_start(d_idxs.ap(), idxs[:])
                d_win2 = nc.dram_tensor(
                    "d_win2", [128, JT], i32, kind="ExternalOutput"
                )
                nc.sync.dma_start(d_win2.ap(), win2[:])

    nc.compile()
    return nc


_NC_CACHE = None


def _get_nc():
    global _NC_CACHE
    if _NC_CACHE is None:
        _NC_CACHE = build_bass()
    return _NC_CACHE


def _base_p():
    p = np.arange(128)
    return (p % 16) * MC + (p // 16) * JT


def _consts():
    base = _base_p()
    am0 = (base[:, None] + np.arange(JT)[None, :]).astype(np.int32)
    w3 = np.tile(
        np.array([D * D, D, 1], dtype=np.float32)[None, :], (128, 1)
    )
    ce = np.tile(np.arange(64, dtype=np.int32)[None, :], (128, 1))
    return am0, w3, ce


# atom id for result slot (p, j): ATOM_ORDER[p*JT + j] = base(p) + j
ATOM_ORDER = (_base_p()[:, None] + np.arange(JT)[None, :]).reshape(-1)


def unscramble(res):
    """res: [128, JT, F] device result -> [F, AC] in atom order."""
    oc = np.empty((F, AC), dtype=np.float32)
    oc[:, ATOM_ORDER] = np.asarray(res).transpose(2, 0, 1).reshape(F, AC)
    return oc


def make_in_maps(volume, coords, num_atoms):
    import ml_dtypes

    am0, w3, ce = _consts()
    vols = {}
    in_maps = []
    for c in range(N_CORES):
        b, h = c // 2, c % 2
        if b not in vols:
            v = volume[b].reshape(F, NROWS, 64).transpose(1, 0, 2)
            vols[b] = np.ascontiguousarray(
                v.astype(ml_dtypes.bfloat16)
            ).reshape(-1)
        in_maps.append(
            {
                "vol": vols[b],
                "crd": np.ascontiguousarray(
                    coords[b, h * 3 * AC : (h + 1) * 3 * AC]
                ),
                # am0 holds core-LOCAL atom ids; shift the mask threshold
                "nat": np.full((128,), int(num_atoms[b]) - h * AC, dtype=np.int32),
                "am0": am0,
                "w3c": w3,
                "cec": ce,
            }
        )
    return in_maps


def kernel(volume, coords, num_atoms):
    volume = np.asarray(volume, dtype=np.float32)
    coords = np.asarray(coords, dtype=np.float32)
    num_atoms = np.asarray(num_atoms, dtype=np.int32)

    nc = _get_nc()
    in_maps = make_in_maps(volume, coords, num_atoms)
    r = run_bass_kernel_spmd(nc, in_maps, core_ids=list(range(N_CORES)))

    out = np.empty((B, F, A), dtype=np.float32)
    for c, res in enumerate(r.results):
        b, h = c // 2, c % 2
        out[b, :, h * AC : (h + 1) * AC] = unscramble(res["out"])
    return out
